# revision 19
# baseline (speedup 1.0000x reference)
"""CVAE loss kernel for Trainium2 (8 NeuronCores, data-parallel over batch).

Reference computation (per problem):
  ep = concat([events[:, :-1], events[:, 1:]], -1)          # [B, L-1, 2H]
  z_mean      = ep @ Wzm + bzm                              # [B, L-1, G]
  z_log_var   = ep @ Wzv + bzv
  ce = concat([ep, contexts], -1)                           # [B, L-1, 3H]
  q_z_mean    = ce @ Wqm + bqm
  q_z_log_var = ce @ Wqv + bqv
  kl_loss     = mean_{b,l} 0.5 * sum_G(qlv - zlv + (exp(zlv) + (zm-qm)^2)*exp(-qlv) - 1)

Strategy:
  - Shard B=16 across 8 cores (2 batches each), replicate the small weights.
  - Never materialize ep/ce: z_mean = events[:-1] @ Wzm[:H] + events[1:] @ Wzm[H:],
    realized as a +1 column shift of the transposed events tile in the moving
    operand of the matmul.
  - float32r matmuls (TF32-like, ~1.5e-4 rel err) run at full PE rate for
    moving free dim >= 256; inputs are rounded to f32r by cast-on-DMA (SWDGE).
  - Per (batch, row-tile of 512 pair rows): PE-transpose X/C 128x128 blocks into
    [H-chunk, rows] tiles, 80 weight-stationary matmuls accumulate the four
    outputs in PSUM [G, rows]; ScalarE drains + bias (+ exp for KL); VectorE
    does the KL elementwise + per-partition reductions; outputs are
    PE-transposed back to [rows, G] and DMA'd out. KL partial sums per core are
    combined on the host.
"""

import sys

for _p in ("/opt/trn_rl_repo",):
    if _p not in sys.path:
        sys.path.insert(0, _p)

import numpy as np
from contextlib import ExitStack

from concourse import bacc, mybir
from concourse.tile import TileContext
from concourse.bass_utils import run_bass_kernel_spmd

B, L, H, G = 16, 1024, 1024, 128
NCORES = 8
BPC = B // NCORES          # batches per core
NPAIR = L - 1              # 1023 adjacent pairs per batch
KC = H // 128              # 8 contraction chunks of 128
RT = 512                   # row-tile (pair rows per tile)
F32 = mybir.dt.float32
F32R = mybir.dt.float32r
AF = mybir.ActivationFunctionType
ALU = mybir.AluOpType

_CACHE = {}


def _build():
    nc = bacc.Bacc("TRN2", target_bir_lowering=False, debug=False)

    ev_d = nc.dram_tensor("ev", [BPC, L, H], F32, kind="ExternalInput").ap()
    cx_d = nc.dram_tensor("cx", [BPC, NPAIR, H], F32, kind="ExternalInput").ap()
    wzm_d = nc.dram_tensor("wzm", [2 * H, G], F32, kind="ExternalInput").ap()
    wzv_d = nc.dram_tensor("wzv", [2 * H, G], F32, kind="ExternalInput").ap()
    wqm_d = nc.dram_tensor("wqm", [3 * H, G], F32, kind="ExternalInput").ap()
    wqv_d = nc.dram_tensor("wqv", [3 * H, G], F32, kind="ExternalInput").ap()
    bzm_d = nc.dram_tensor("bzm", [G, 1], F32, kind="ExternalInput").ap()
    bzv_d = nc.dram_tensor("bzv", [G, 1], F32, kind="ExternalInput").ap()
    bqm_d = nc.dram_tensor("bqm", [G, 1], F32, kind="ExternalInput").ap()
    bqv_d = nc.dram_tensor("bqv", [G, 1], F32, kind="ExternalInput").ap()
    id_d = nc.dram_tensor("ident", [128, 128], F32, kind="ExternalInput").ap()

    zm_d = nc.dram_tensor("zm", [BPC, NPAIR, G], F32, kind="ExternalOutput").ap()
    zv_d = nc.dram_tensor("zv", [BPC, NPAIR, G], F32, kind="ExternalOutput").ap()
    qm_d = nc.dram_tensor("qm", [BPC, NPAIR, G], F32, kind="ExternalOutput").ap()
    qv_d = nc.dram_tensor("qv", [BPC, NPAIR, G], F32, kind="ExternalOutput").ap()
    kl_d = nc.dram_tensor("klsum", [1, 1], F32, kind="ExternalOutput").ap()

    with TileContext(nc) as tc, ExitStack() as ctx:
        const = ctx.enter_context(tc.tile_pool(name="const", bufs=1))
        xp = ctx.enter_context(tc.tile_pool(name="xp", bufs=2))
        tp = ctx.enter_context(tc.tile_pool(name="tp", bufs=2))
        op = ctx.enter_context(tc.tile_pool(name="op", bufs=2))
        klp = ctx.enter_context(tc.tile_pool(name="klp", bufs=1))
        stp = ctx.enter_context(tc.tile_pool(name="stp", bufs=4))
        ps_acc = ctx.enter_context(tc.tile_pool(name="ps_acc", bufs=4, space="PSUM"))
        ps_tr = ctx.enter_context(tc.tile_pool(name="ps_tr", bufs=2, space="PSUM"))
        ps_ot = ctx.enter_context(tc.tile_pool(name="ps_ot", bufs=1, space="PSUM"))
        ps_mini = ctx.enter_context(tc.tile_pool(name="ps_mini", bufs=1, space="PSUM"))

        # ---- startup-critical-path order: identity, first tile's X, the
        # first weight matrix, then everything else ----
        ident_r = const.tile([128, 128], F32R, tag="ident_r")
        nc.gpsimd.dma_start(ident_r[:], id_d[:])

        first_x = xp.tile([128, 4, H], F32R, tag="x")
        nc.gpsimd.dma_start(
            first_x[:], ev_d[0, 0:RT, :].rearrange("(r p) h -> p r h", p=128)
        )
        wzm_sb = const.tile([128, 2 * KC, G], F32R, tag="wzm")
        nc.gpsimd.dma_start(wzm_sb[:], wzm_d.rearrange("(c p) g -> p c g", p=128))
        first_c = xp.tile([128, 4, H], F32R, tag="c")
        nc.gpsimd.dma_start(
            first_c[:], cx_d[0, 0:RT, :].rearrange("(r p) h -> p r h", p=128)
        )
        wzv_sb = const.tile([128, 2 * KC, G], F32R, tag="wzv")
        wqm_sb = const.tile([128, 3 * KC, G], F32R, tag="wqm")
        wqv_sb = const.tile([128, 3 * KC, G], F32R, tag="wqv")
        nc.gpsimd.dma_start(wzv_sb[:], wzv_d.rearrange("(c p) g -> p c g", p=128))
        nc.gpsimd.dma_start(wqm_sb[:], wqm_d.rearrange("(c p) g -> p c g", p=128))
        nc.gpsimd.dma_start(wqv_sb[:], wqv_d.rearrange("(c p) g -> p c g", p=128))

        ident_f = const.tile([128, 128], F32, tag="ident_f")
        nc.sync.dma_start(ident_f[:], id_d[:])

        bzm_sb = const.tile([G, 1], F32, tag="bzm")
        bzv_sb = const.tile([G, 1], F32, tag="bzv")
        bqm_sb = const.tile([G, 1], F32, tag="bqm")
        bqv_sb = const.tile([G, 1], F32, tag="bqv")
        nc.sync.dma_start(bzm_sb[:], bzm_d[:])
        nc.sync.dma_start(bzv_sb[:], bzv_d[:])
        nc.sync.dma_start(bqm_sb[:], bqm_d[:])
        nc.sync.dma_start(bqv_sb[:], bqv_d[:])
        nbqv_sb = const.tile([G, 1], F32, tag="nbqv")
        nc.vector.tensor_scalar_mul(nbqv_sb[:], bqv_sb[:], -1.0)

        ones_sb = const.tile([128, 1], F32, tag="ones")
        nc.gpsimd.memset(ones_sb[:], 1.0)
        klacc = const.tile([128, 1], F32, tag="klacc")
        nc.gpsimd.memset(klacc[:], 0.0)

        for b in range(BPC):
            for t in range(2):
                l0 = t * RT
                nr = RT if t == 0 else NPAIR - RT  # 512 or 511

                # ---- load X (events rows l0..l0+511) and C tiles ----
                if b == 0 and t == 0:
                    x_sb, c_sb = first_x, first_c
                else:
                    x_sb = xp.tile([128, 4, H], F32R, tag="x")
                    nc.gpsimd.dma_start(
                        x_sb[:],
                        ev_d[b, l0 : l0 + RT, :].rearrange("(r p) h -> p r h", p=128),
                    )
                    c_sb = xp.tile([128, 4, H], F32R, tag="c")
                if b == 0 and t == 0:
                    pass
                elif t == 0:
                    nc.gpsimd.dma_start(
                        c_sb[:],
                        cx_d[b, l0 : l0 + RT, :].rearrange("(r p) h -> p r h", p=128),
                    )
                else:
                    nc.gpsimd.dma_start(
                        c_sb[:, 0:3, :],
                        cx_d[b, l0 : l0 + 384, :].rearrange("(r p) h -> p r h", p=128),
                    )
                    nc.gpsimd.dma_start(c_sb[0:127, 3, :], cx_d[b, l0 + 384 : NPAIR, :])
                    # engines can't start at partition 127; fill the missing
                    # row with a copy of the last context row (never read back)
                    nc.gpsimd.dma_start(c_sb[127:128, 3, :], cx_d[b, NPAIR - 1 : NPAIR, :])

                # ---- transpose into [H-chunk, rows] ----
                xt = tp.tile([128, KC, 520], F32R, tag="xt")
                ct = tp.tile([128, KC, 512], F32R, tag="ct")
                for k in range(KC):
                    hs = slice(k * 128, (k + 1) * 128)
                    tr = ps_tr.tile([128, 512], F32R, tag="tr")
                    for rb in range(4):
                        nc.tensor.transpose(
                            tr[:, rb * 128 : (rb + 1) * 128], x_sb[:, rb, hs], ident_r[:]
                        )
                    nc.vector.tensor_copy(xt[:, k, 0:512], tr[:])
                    tr = ps_tr.tile([128, 512], F32R, tag="tr")
                    for rb in range(4):
                        # t=1 rb=3 has only 127 valid rows; transpose the full
                        # 128 block anyway (ISA wants 32-multiples) — the
                        # garbage last column is never copied or read.
                        nc.tensor.transpose(
                            tr[:, rb * 128 : (rb + 1) * 128], c_sb[:, rb, hs], ident_r[:]
                        )
                    nc.vector.tensor_copy(ct[:, k, 0:512], tr[:])
                # boundary row (l0+512) fills xt column 512 for the shifted
                # matmul; at t=1 there is no such row — load row RT again, the
                # junk lands in PSUM col 511 which every consumer slices away.
                nc.gpsimd.dma_start(
                    xt[:, :, 512], ev_d[b, RT, :].rearrange("(c p) -> p c", p=128)
                )

                # ---- matmuls: accumulate the four outputs in PSUM [G, nr] ----
                zm_ps = ps_acc.tile([G, 512], F32, tag="acc")
                zv_ps = ps_acc.tile([G, 512], F32, tag="acc")
                qm_ps = ps_acc.tile([G, 512], F32, tag="acc")
                qv_ps = ps_acc.tile([G, 512], F32, tag="acc")
                # matmuls always run the full 512 columns (f32r wants aligned
                # N); at t=1 column 511 of PSUM is garbage and every consumer
                # below slices [0:nr].
                for k in range(KC):
                    al = xt[:, k, 0:512]       # rows l0+j
                    sh = xt[:, k, 1:513]       # rows l0+j+1
                    cx_al = ct[:, k, 0:512]
                    st, sp = k == 0, k == KC - 1
                    nc.tensor.matmul(zm_ps[:], wzm_sb[:, k, :], al, start=st, stop=False)
                    nc.tensor.matmul(zm_ps[:], wzm_sb[:, KC + k, :], sh, start=False, stop=sp)
                    nc.tensor.matmul(zv_ps[:], wzv_sb[:, k, :], al, start=st, stop=False)
                    nc.tensor.matmul(zv_ps[:], wzv_sb[:, KC + k, :], sh, start=False, stop=sp)
                    nc.tensor.matmul(qm_ps[:], wqm_sb[:, k, :], al, start=st, stop=False)
                    nc.tensor.matmul(qm_ps[:], wqm_sb[:, KC + k, :], sh, start=False, stop=False)
                    nc.tensor.matmul(qm_ps[:], wqm_sb[:, 2 * KC + k, :], cx_al, start=False, stop=sp)
                    nc.tensor.matmul(qv_ps[:], wqv_sb[:, k, :], al, start=st, stop=False)
                    nc.tensor.matmul(qv_ps[:], wqv_sb[:, KC + k, :], sh, start=False, stop=False)
                    nc.tensor.matmul(qv_ps[:], wqv_sb[:, 2 * KC + k, :], cx_al, start=False, stop=sp)

                # ---- drain + bias (ScalarE), KL elementwise ----
                zm_sb = op.tile([G, 512], F32, tag="zm")
                zv_sb = op.tile([G, 512], F32, tag="zv")
                qm_sb = op.tile([G, 512], F32, tag="qm")
                qv_sb = op.tile([G, 512], F32, tag="qv")
                a1 = klp.tile([G, 1], F32, tag="a1")
                nc.scalar.activation(zm_sb[:], zm_ps[:], AF.Identity, bias=bzm_sb[:])
                nc.scalar.activation(zv_sb[:], zv_ps[:], AF.Identity, bias=bzv_sb[:])
                nc.scalar.activation(qm_sb[:], qm_ps[:], AF.Identity, bias=bqm_sb[:])
                nc.scalar.activation(qv_sb[:], qv_ps[:], AF.Identity, bias=bqv_sb[:])
                # e2 = exp(-(qv_ps + bqv)) straight from PSUM
                # NOTE: tensor_tensor_reduce faults the exec unit on this
                # HW (NRT status 101); use mul + reduce instead.
                e2 = klp.tile([G, 512], F32, tag="e2")
                nc.scalar.activation(e2[:, 0:nr], qv_ps[:, 0:nr], AF.Exp, bias=nbqv_sb[:], scale=-1.0)
                d1 = klp.tile([G, 512], F32, tag="d1")
                nc.vector.tensor_sub(d1[:, 0:nr], zv_sb[:, 0:nr], qv_sb[:, 0:nr])
                e1 = klp.tile([G, 512], F32, tag="e1")
                nc.scalar.activation(e1[:, 0:nr], d1[:, 0:nr], AF.Exp, accum_out=a1[:])
                d2 = klp.tile([G, 512], F32, tag="d2")
                nc.vector.tensor_sub(d2[:, 0:nr], zm_sb[:, 0:nr], qm_sb[:, 0:nr])
                t2 = klp.tile([G, 512], F32, tag="t2")
                nc.vector.tensor_mul(t2[:, 0:nr], d2[:, 0:nr], e2[:, 0:nr])
                junk = klp.tile([G, 512], F32, tag="junk")
                nc.vector.tensor_mul(junk[:, 0:nr], d2[:, 0:nr], t2[:, 0:nr])
                tsum = klp.tile([G, 1], F32, tag="tsum")
                nc.vector.tensor_reduce(tsum[:], junk[:, 0:nr], axis=mybir.AxisListType.X, op=ALU.add)
                s1 = klp.tile([G, 1], F32, tag="s1")
                nc.vector.tensor_reduce(s1[:], d1[:, 0:nr], axis=mybir.AxisListType.X, op=ALU.add)
                s2 = klp.tile([G, 1], F32, tag="s2")
                nc.vector.tensor_sub(s2[:], a1[:], s1[:])  # sum(e1) + sum(qlv - zlv)
                s3 = klp.tile([G, 1], F32, tag="s3")
                nc.vector.tensor_add(s3[:], s2[:], tsum[:])
                nc.vector.tensor_add(klacc[:], klacc[:], s3[:])

                # ---- transpose outputs back to [rows, G] and store ----
                for o_sb, o_d in ((zm_sb, zm_d), (zv_sb, zv_d), (qm_sb, qm_d), (qv_sb, qv_d)):
                    ot = ps_ot.tile([128, 4, G], F32, tag="ot")
                    for rb in range(4):
                        # full 128 block even when only 127 rows are valid
                        # (t=1 rb=3); the extra row is never DMA'd out.
                        nc.tensor.transpose(
                            ot[:, rb, :], o_sb[:, rb * 128 : (rb + 1) * 128], ident_f[:]
                        )
                    stg = stp.tile([128, 4, G], F32, tag="stg")
                    nc.scalar.copy(stg[:], ot[:])
                    if t == 0:
                        nc.sync.dma_start(
                            o_d[b, l0 : l0 + RT, :].rearrange("(r p) g -> p r g", p=128),
                            stg[:],
                        )
                    else:
                        nc.sync.dma_start(
                            o_d[b, l0 : l0 + 384, :].rearrange("(r p) g -> p r g", p=128),
                            stg[:, 0:3, :],
                        )
                        nc.sync.dma_start(o_d[b, l0 + 384 : NPAIR, :], stg[0:127, 3, :])

        # ---- final cross-partition KL reduction ----
        kl_ps = ps_mini.tile([1, 1], F32, tag="mini")
        nc.tensor.matmul(kl_ps[:], klacc[:], ones_sb[:], start=True, stop=True)
        kl_sb = const.tile([1, 1], F32, tag="klout")
        nc.vector.tensor_copy(kl_sb[:], kl_ps[:])
        nc.sync.dma_start(kl_d[:], kl_sb[:])

    nc.compile()
    return nc


def _get_nc():
    if "nc" not in _CACHE:
        _CACHE["nc"] = _build()
    return _CACHE["nc"]


def kernel(events, contexts, Wzm, bzm, Wzv, bzv, Wqm, bqm, Wqv, bqv):
    events = np.ascontiguousarray(np.asarray(events, dtype=np.float32))
    contexts = np.ascontiguousarray(np.asarray(contexts, dtype=np.float32))
    Wzm = np.ascontiguousarray(np.asarray(Wzm, dtype=np.float32))
    Wzv = np.ascontiguousarray(np.asarray(Wzv, dtype=np.float32))
    Wqm = np.ascontiguousarray(np.asarray(Wqm, dtype=np.float32))
    Wqv = np.ascontiguousarray(np.asarray(Wqv, dtype=np.float32))
    bzm = np.asarray(bzm, dtype=np.float32).reshape(G, 1).copy()
    bzv = np.asarray(bzv, dtype=np.float32).reshape(G, 1).copy()
    bqm = np.asarray(bqm, dtype=np.float32).reshape(G, 1).copy()
    bqv = np.asarray(bqv, dtype=np.float32).reshape(G, 1).copy()
    ident = np.eye(128, dtype=np.float32)

    nc = _get_nc()
    in_maps = []
    for c in range(NCORES):
        in_maps.append(
            {
                "ev": np.ascontiguousarray(events[c * BPC : (c + 1) * BPC]),
                "cx": np.ascontiguousarray(contexts[c * BPC : (c + 1) * BPC]),
                "wzm": Wzm, "wzv": Wzv, "wqm": Wqm, "wqv": Wqv,
                "bzm": bzm, "bzv": bzv, "bqm": bqm, "bqv": bqv,
                "ident": ident,
            }
        )
    res = run_bass_kernel_spmd(nc, in_maps, core_ids=list(range(NCORES)))

    zm = np.concatenate([r["zm"] for r in res.results], axis=0)
    zv = np.concatenate([r["zv"] for r in res.results], axis=0)
    qm = np.concatenate([r["qm"] for r in res.results], axis=0)
    qv = np.concatenate([r["qv"] for r in res.results], axis=0)
    tot = float(B * NPAIR)
    ksum = float(sum(float(r["klsum"][0, 0]) for r in res.results))
    kl = np.array([0.5 * (ksum - tot * G) / tot], dtype=np.float32)
    return (zm, zv, qm, qv, kl)
